# revision 10
# baseline (speedup 1.0000x reference)
"""KVGather Trainium2 kernel.

Problem: out[n, i, k] = r_weight[n, i, k] * kv[n, r_idx[n, i, k]]
  r_idx:    (16, 64, 8)  int64, values in [0, 64)
  r_weight: (16, 64, 8)  float32
  kv:       (16, 64, 64, 128) float32
  out:      (16, 64, 8, 64, 128) float32

Strategy: data-parallel over batch n across 8 NeuronCores (2 batches/core).
Per core: 4 MB kv in, 32 MB out -> HBM-write-bound (~100us at ~358 GB/s).

The gather+scale is reformulated as a one-hot matmul so all device
addressing is static:
  - Host splits f32 kv into kv_hi + kv_lo (bf16 pair, exact to ~2^-18).
  - Host builds binary selection matrices S (bf16, entries {0,1}): for
    output-slot chunk j, S[j][r, o] = 1 iff slot o of chunk j routes
    region r. Both batches of a core are packed into the 128-partition
    contraction dim (batch0 regions -> partitions 0..63, batch1 -> 64..127).
  - PE: psum = S.T @ kv_hi + S.T @ kv_lo  (two accumulating bf16 matmuls)
    = gathered kv rows, f32 in PSUM.
  - DVE/ACT drain PSUM -> SBUF fused with the full-precision f32 weight
    multiply (tensor_scalar_mul with a per-partition scalar).
  - HWDGE DMA streams staging -> out DRAM in ~512KB transfers.
"""

import sys

for _p in ("/opt/trn_rl_repo",):
    if _p not in sys.path:
        sys.path.insert(0, _p)

import numpy as np
import ml_dtypes

from concourse import bass, bacc, tile
from concourse import mybir
from concourse.bass_utils import run_bass_kernel_spmd

# Problem constants (hardcoded per contract)
N, P2, TOPK, W2, C_KV = 16, 64, 8, 64, 128
N_CORES = 8
B = N // N_CORES            # batches per core = 2
SLOTS = P2 * TOPK           # 512 output slots per batch
F = W2 * C_KV               # 8192 elements per region
N_CHUNK = 2 * SLOTS // 128  # 8 chunks of 128 packed output slots
FC = 4                      # kv f-dim split for load/compute overlap
F_PER_FC = F // FC          # 2048
T_PER_FC = F_PER_FC // 512  # 4 psum tiles of 512 per f-chunk

_cached = {}


def _build_program():
    """Build the (input-independent) Bass program once."""
    if "nc" in _cached:
        return _cached["nc"]

    bf16 = mybir.dt.bfloat16
    f32 = mybir.dt.float32

    nc = bacc.Bacc()

    # Per-core inputs. kv planes laid out [128, FC, F_PER_FC]:
    # partition p = (batch b = p//64, region r = p%64); free = region elems.
    kv_hi_d = nc.dram_tensor("kv_hi", [128, FC, F_PER_FC], bf16, kind="ExternalInput")
    kv_lo_d = nc.dram_tensor("kv_lo", [128, FC, F_PER_FC], bf16, kind="ExternalInput")
    # Selection matrices, r-major so the SBUF-bound DMA is contiguous:
    # s_d[r, j, o] = 1 iff chunk j's packed slot o routes region-partition r.
    s_d = nc.dram_tensor("sel", [128, N_CHUNK, 128], bf16, kind="ExternalInput")
    # w_d[o, j] = f32 weight of chunk j's packed slot o.
    w_d = nc.dram_tensor("wgt", [128, N_CHUNK], f32, kind="ExternalInput")
    # Output, shaped so every DMA slice is contiguous per slot:
    # [b, j, o_in_batch(64), F]
    out_d = nc.dram_tensor("out", [B, N_CHUNK, 64, F], f32, kind="ExternalOutput")

    with tile.TileContext(nc) as tc:
        with (
            tc.tile_pool(name="const", bufs=1) as const_pool,
            tc.tile_pool(name="kv", bufs=1) as kv_pool,
            tc.tile_pool(name="stage", bufs=4) as stage_pool,
            tc.tile_pool(name="psum", bufs=8, space=bass.MemorySpace.PSUM) as psum_pool,
        ):
            s_sb = const_pool.tile([128, N_CHUNK, 128], bf16, tag="sel")
            w_sb = const_pool.tile([128, N_CHUNK], f32, tag="wgt")
            nc.sync.dma_start(out=s_sb[:], in_=s_d[:])
            nc.sync.dma_start(out=w_sb[:], in_=w_d[:])

            # kv planes as FC separate tiles so compute can start after the
            # first f-chunk lands.
            kv_hi_sb = []
            kv_lo_sb = []
            for fc in range(FC):
                th = kv_pool.tile([128, F_PER_FC], bf16, tag=f"kvh{fc}")
                tl = kv_pool.tile([128, F_PER_FC], bf16, tag=f"kvl{fc}")
                nc.sync.dma_start(out=th[:], in_=kv_hi_d[:, fc, :])
                nc.sync.dma_start(out=tl[:], in_=kv_lo_d[:, fc, :])
                kv_hi_sb.append(th)
                kv_lo_sb.append(tl)

            group_i = 0
            for fc in range(FC):
                for j in range(N_CHUNK):
                    # One engine per (fc, j) group keeps the per-instruction
                    # sync-wait count within the ISA budget; groups alternate
                    # DVE/ACT so both engines split the drain work.
                    use_dve = group_i % 2 == 0
                    group_i += 1
                    stage = stage_pool.tile([128, F_PER_FC], f32, tag="stage")
                    for t in range(T_PER_FC):
                        ps = psum_pool.tile([128, 512], f32, tag="ps")
                        lhsT = s_sb[:, j, :]
                        nc.tensor.matmul(
                            ps[:],
                            lhsT,
                            kv_hi_sb[fc][:, t * 512 : (t + 1) * 512],
                            start=True,
                            stop=False,
                        )
                        nc.tensor.matmul(
                            ps[:],
                            lhsT,
                            kv_lo_sb[fc][:, t * 512 : (t + 1) * 512],
                            start=False,
                            stop=True,
                        )
                        # Drain fused with the f32 weight multiply.
                        if use_dve:
                            nc.vector.tensor_mul(
                                stage[:, t * 512 : (t + 1) * 512],
                                ps[:],
                                w_sb[:, j : j + 1].broadcast_to([128, 512]),
                            )
                        else:
                            nc.scalar.activation(
                                stage[:, t * 512 : (t + 1) * 512],
                                ps[:],
                                mybir.ActivationFunctionType.Copy,
                                scale=w_sb[:, j : j + 1],
                            )
                    # Single fan-out store: (2,64,F_PER_FC) <-> (128,F_PER_FC)
                    nc.sync.dma_start(
                        out=out_d[:, j, :, fc * F_PER_FC : (fc + 1) * F_PER_FC],
                        in_=stage[:],
                    )

    nc.compile()
    _cached["nc"] = nc
    return nc


def _prep_inputs(r_idx, r_weight, kv):
    """Shard + transform host inputs into per-core in_maps."""
    r_idx = np.asarray(r_idx).astype(np.int64)
    r_weight = np.asarray(r_weight).astype(np.float32)
    kv = np.asarray(kv).astype(np.float32)

    # hi/lo bf16 split of kv (exact to ~2^-18 relative)
    kv_hi = kv.astype(ml_dtypes.bfloat16)
    kv_lo = (kv - kv_hi.astype(np.float32)).astype(ml_dtypes.bfloat16)

    in_maps = []
    for m in range(N_CORES):
        bsl = slice(m * B, (m + 1) * B)
        # [B, P2, W2, C] -> [B*P2(=128 partitions), F] -> [128, FC, F_PER_FC]
        kvh = np.ascontiguousarray(
            kv_hi[bsl].reshape(128, FC, F_PER_FC)
        )
        kvl = np.ascontiguousarray(
            kv_lo[bsl].reshape(128, FC, F_PER_FC)
        )

        idx = r_idx[bsl].reshape(B, SLOTS)        # [2, 512] region ids
        wgt = r_weight[bsl].reshape(B, SLOTS)     # [2, 512] f32

        # S[r, j, o]: packed slot o of chunk j -> batch b=o//64,
        # slot index j*64 + (o%64); routes region idx[b, slot].
        S = np.zeros((128, N_CHUNK, 128), dtype=ml_dtypes.bfloat16)
        W = np.zeros((128, N_CHUNK), dtype=np.float32)
        for j in range(N_CHUNK):
            for o in range(128):
                b = o // 64
                slot = j * 64 + (o % 64)
                r = int(idx[b, slot])
                S[b * 64 + r, j, o] = 1.0
                W[o, j] = wgt[b, slot]

        in_maps.append({"kv_hi": kvh, "kv_lo": kvl, "sel": S, "wgt": W})
    return in_maps


def _ensure_ntff_hook():
    """The agent image's antenv lacks axon_hooks, so the boot-time NTFF
    hook registration silently no-ops. Recreate the module and register
    the ctypes hook so trace=True yields exec_time_ns."""
    import types
    import antenv

    if "antenv.axon_hooks" in sys.modules:
        return
    mod = types.ModuleType("antenv.axon_hooks")
    _state = {"hook": None}
    mod.set_axon_ntff_profile_hook = lambda h: _state.__setitem__("hook", h)
    mod.get_axon_ntff_profile_hook = lambda: _state["hook"]
    sys.modules["antenv.axon_hooks"] = mod
    antenv.axon_hooks = mod
    try:
        if "/root/.axon_site" not in sys.path:
            sys.path.insert(0, "/root/.axon_site")
        from trn_agent_boot.trn_boot import _ntff_profile_via_ctypes

        hook = _ntff_profile_via_ctypes("/opt/axon/libaxon_pjrt.so")
        if hook is not None:
            mod.set_axon_ntff_profile_hook(hook)
    except Exception:
        pass


def kernel(r_idx, r_weight, kv, _trace=False, _trace_kwargs=None):
    if _trace:
        _ensure_ntff_hook()
    nc = _build_program()
    in_maps = _prep_inputs(r_idx, r_weight, kv)
    res = run_bass_kernel_spmd(
        nc,
        in_maps,
        core_ids=list(range(N_CORES)),
        trace=_trace,
        **(_trace_kwargs or {}),
    )
    out = np.empty((N, P2, TOPK, W2, C_KV), dtype=np.float32)
    for m in range(N_CORES):
        o = res.results[m]["out"]  # [B, N_CHUNK, 64, F]
        # (j, o64) enumerate slots in order; (slot) = (i, k); F = (w2, c)
        o = o.reshape(B, SLOTS, F).reshape(B, P2, TOPK, W2, C_KV)
        out[m * B : (m + 1) * B] = o
    if _trace:
        return out, res
    return out


if __name__ == "__main__":
    rng = np.random.default_rng(0)
    r_idx = rng.integers(0, P2, (N, P2, TOPK)).astype(np.int64)
    r_weight = rng.random((N, P2, TOPK), dtype=np.float32)
    kv = rng.standard_normal((N, P2, W2, C_KV), dtype=np.float32)
    out = kernel(r_idx, r_weight, kv)
    # local reference
    bidx = np.arange(N)[:, None, None]
    exp = r_weight[..., None, None] * kv[bidx, r_idx]
    err = np.abs(out - exp).max() / (np.abs(exp).max() + 1e-30)
    print("abs-rel err:", err)


# revision 11
# speedup vs baseline: 2.7675x; 2.7675x over previous
"""KVGather Trainium2 kernel.

Problem: out[n, i, k] = r_weight[n, i, k] * kv[n, r_idx[n, i, k]]
  r_idx:    (16, 64, 8)  int64, values in [0, 64)
  r_weight: (16, 64, 8)  float32
  kv:       (16, 64, 64, 128) float32
  out:      (16, 64, 8, 64, 128) float32

Strategy: data-parallel over batch n across 8 NeuronCores (2 batches/core).
Per core: 4 MB kv in, 32 MB out -> HBM-write-bound (~100us at ~358 GB/s).

The gather+scale is reformulated as a one-hot matmul so all device
addressing is static:
  - Host splits f32 kv into kv_hi + kv_lo (bf16 pair, exact to ~2^-18).
  - Host builds binary selection matrices S (bf16, entries {0,1}): for
    output-slot chunk j, S[j][r, o] = 1 iff slot o of chunk j routes
    region r. Both batches of a core are packed into the 128-partition
    contraction dim (batch0 regions -> partitions 0..63, batch1 -> 64..127).
  - PE: psum = S.T @ kv_hi + S.T @ kv_lo  (two accumulating bf16 matmuls)
    = gathered kv rows, f32 in PSUM.
  - DVE/ACT drain PSUM -> SBUF fused with the full-precision f32 weight
    multiply (tensor_scalar_mul with a per-partition scalar).
  - HWDGE DMA streams staging -> out DRAM in ~512KB transfers.
"""

import sys

for _p in ("/opt/trn_rl_repo",):
    if _p not in sys.path:
        sys.path.insert(0, _p)

import numpy as np
import ml_dtypes

from concourse import bass, bacc, tile
from concourse import mybir
from concourse.bass_utils import run_bass_kernel_spmd

# Problem constants (hardcoded per contract)
N, P2, TOPK, W2, C_KV = 16, 64, 8, 64, 128
N_CORES = 8
B = N // N_CORES            # batches per core = 2
SLOTS = P2 * TOPK           # 512 output slots per batch
F = W2 * C_KV               # 8192 elements per region
N_CHUNK = 2 * SLOTS // 128  # 8 chunks of 128 packed output slots
FC = 4                      # kv f-dim split for load/compute overlap
F_PER_FC = F // FC          # 2048
T_PER_FC = F_PER_FC // 512  # 4 psum tiles of 512 per f-chunk

_cached = {}


def _build_program():
    """Build the (input-independent) Bass program once."""
    if "nc" in _cached:
        return _cached["nc"]

    bf16 = mybir.dt.bfloat16
    f32 = mybir.dt.float32

    nc = bacc.Bacc()

    # Per-core inputs. kv planes laid out [128, FC, F_PER_FC]:
    # partition p = (batch b = p//64, region r = p%64); free = region elems.
    kv_hi_d = nc.dram_tensor("kv_hi", [128, FC, F_PER_FC], bf16, kind="ExternalInput")
    kv_lo_d = nc.dram_tensor("kv_lo", [128, FC, F_PER_FC], bf16, kind="ExternalInput")
    # Selection matrices, r-major so the SBUF-bound DMA is contiguous:
    # s_d[r, j, o] = 1 iff chunk j's packed slot o routes region-partition r.
    s_d = nc.dram_tensor("sel", [128, N_CHUNK, 128], bf16, kind="ExternalInput")
    # w_d[o, j] = f32 weight of chunk j's packed slot o.
    w_d = nc.dram_tensor("wgt", [128, N_CHUNK], f32, kind="ExternalInput")
    # Output, shaped so every DMA slice is contiguous per slot:
    # [b, j, o_in_batch(64), F]
    out_d = nc.dram_tensor("out", [B, N_CHUNK, 64, F], f32, kind="ExternalOutput")

    with tile.TileContext(nc) as tc:
        with (
            tc.tile_pool(name="const", bufs=1) as const_pool,
            tc.tile_pool(name="kv", bufs=1) as kv_pool,
            tc.tile_pool(name="stage", bufs=4) as stage_pool,
            tc.tile_pool(name="psum", bufs=8, space=bass.MemorySpace.PSUM) as psum_pool,
        ):
            s_sb = const_pool.tile([128, N_CHUNK, 128], bf16, tag="sel")
            w_sb = const_pool.tile([128, N_CHUNK], f32, tag="wgt")
            nc.gpsimd.dma_start(out=s_sb[:], in_=s_d[:])
            nc.gpsimd.dma_start(out=w_sb[:], in_=w_d[:])

            # kv planes as FC separate tiles so compute can start after the
            # first f-chunk lands.
            kv_hi_sb = []
            kv_lo_sb = []
            for fc in range(FC):
                th = kv_pool.tile([128, F_PER_FC], bf16, tag=f"kvh{fc}")
                tl = kv_pool.tile([128, F_PER_FC], bf16, tag=f"kvl{fc}")
                nc.gpsimd.dma_start(out=th[:], in_=kv_hi_d[:, fc, :])
                nc.gpsimd.dma_start(out=tl[:], in_=kv_lo_d[:, fc, :])
                kv_hi_sb.append(th)
                kv_lo_sb.append(tl)

            group_i = 0
            for fc in range(FC):
                for j in range(N_CHUNK):
                    # One engine per (fc, j) group keeps the per-instruction
                    # sync-wait count within the ISA budget; groups alternate
                    # DVE/ACT so both engines split the drain work.
                    use_dve = group_i % 2 == 0
                    group_i += 1
                    stage = stage_pool.tile([128, F_PER_FC], f32, tag="stage")
                    for t in range(T_PER_FC):
                        ps = psum_pool.tile([128, 512], f32, tag="ps")
                        lhsT = s_sb[:, j, :]
                        nc.tensor.matmul(
                            ps[:],
                            lhsT,
                            kv_hi_sb[fc][:, t * 512 : (t + 1) * 512],
                            start=True,
                            stop=False,
                        )
                        nc.tensor.matmul(
                            ps[:],
                            lhsT,
                            kv_lo_sb[fc][:, t * 512 : (t + 1) * 512],
                            start=False,
                            stop=True,
                        )
                        # Drain fused with the f32 weight multiply.
                        if use_dve:
                            nc.vector.tensor_mul(
                                stage[:, t * 512 : (t + 1) * 512],
                                ps[:],
                                w_sb[:, j : j + 1].broadcast_to([128, 512]),
                            )
                        else:
                            nc.scalar.activation(
                                stage[:, t * 512 : (t + 1) * 512],
                                ps[:],
                                mybir.ActivationFunctionType.Copy,
                                scale=w_sb[:, j : j + 1],
                            )
                    # Single fan-out store: (2,64,F_PER_FC) <-> (128,F_PER_FC)
                    nc.gpsimd.dma_start(
                        out=out_d[:, j, :, fc * F_PER_FC : (fc + 1) * F_PER_FC],
                        in_=stage[:],
                    )

    nc.compile()
    _cached["nc"] = nc
    return nc


def _prep_inputs(r_idx, r_weight, kv):
    """Shard + transform host inputs into per-core in_maps."""
    r_idx = np.asarray(r_idx).astype(np.int64)
    r_weight = np.asarray(r_weight).astype(np.float32)
    kv = np.asarray(kv).astype(np.float32)

    # hi/lo bf16 split of kv (exact to ~2^-18 relative)
    kv_hi = kv.astype(ml_dtypes.bfloat16)
    kv_lo = (kv - kv_hi.astype(np.float32)).astype(ml_dtypes.bfloat16)

    in_maps = []
    for m in range(N_CORES):
        bsl = slice(m * B, (m + 1) * B)
        # [B, P2, W2, C] -> [B*P2(=128 partitions), F] -> [128, FC, F_PER_FC]
        kvh = np.ascontiguousarray(
            kv_hi[bsl].reshape(128, FC, F_PER_FC)
        )
        kvl = np.ascontiguousarray(
            kv_lo[bsl].reshape(128, FC, F_PER_FC)
        )

        idx = r_idx[bsl].reshape(B, SLOTS)        # [2, 512] region ids
        wgt = r_weight[bsl].reshape(B, SLOTS)     # [2, 512] f32

        # S[r, j, o]: packed slot o of chunk j -> batch b=o//64,
        # slot index j*64 + (o%64); routes region idx[b, slot].
        S = np.zeros((128, N_CHUNK, 128), dtype=ml_dtypes.bfloat16)
        W = np.zeros((128, N_CHUNK), dtype=np.float32)
        for j in range(N_CHUNK):
            for o in range(128):
                b = o // 64
                slot = j * 64 + (o % 64)
                r = int(idx[b, slot])
                S[b * 64 + r, j, o] = 1.0
                W[o, j] = wgt[b, slot]

        in_maps.append({"kv_hi": kvh, "kv_lo": kvl, "sel": S, "wgt": W})
    return in_maps


def _ensure_ntff_hook():
    """The agent image's antenv lacks axon_hooks, so the boot-time NTFF
    hook registration silently no-ops. Recreate the module and register
    the ctypes hook so trace=True yields exec_time_ns."""
    import types
    import antenv

    if "antenv.axon_hooks" in sys.modules:
        return
    mod = types.ModuleType("antenv.axon_hooks")
    _state = {"hook": None}
    mod.set_axon_ntff_profile_hook = lambda h: _state.__setitem__("hook", h)
    mod.get_axon_ntff_profile_hook = lambda: _state["hook"]
    sys.modules["antenv.axon_hooks"] = mod
    antenv.axon_hooks = mod
    try:
        if "/root/.axon_site" not in sys.path:
            sys.path.insert(0, "/root/.axon_site")
        from trn_agent_boot.trn_boot import _ntff_profile_via_ctypes

        hook = _ntff_profile_via_ctypes("/opt/axon/libaxon_pjrt.so")
        if hook is not None:
            mod.set_axon_ntff_profile_hook(hook)
    except Exception:
        pass


def kernel(r_idx, r_weight, kv, _trace=False, _trace_kwargs=None):
    if _trace:
        _ensure_ntff_hook()
    nc = _build_program()
    in_maps = _prep_inputs(r_idx, r_weight, kv)
    res = run_bass_kernel_spmd(
        nc,
        in_maps,
        core_ids=list(range(N_CORES)),
        trace=_trace,
        **(_trace_kwargs or {}),
    )
    out = np.empty((N, P2, TOPK, W2, C_KV), dtype=np.float32)
    for m in range(N_CORES):
        o = res.results[m]["out"]  # [B, N_CHUNK, 64, F]
        # (j, o64) enumerate slots in order; (slot) = (i, k); F = (w2, c)
        o = o.reshape(B, SLOTS, F).reshape(B, P2, TOPK, W2, C_KV)
        out[m * B : (m + 1) * B] = o
    if _trace:
        return out, res
    return out


if __name__ == "__main__":
    rng = np.random.default_rng(0)
    r_idx = rng.integers(0, P2, (N, P2, TOPK)).astype(np.int64)
    r_weight = rng.random((N, P2, TOPK), dtype=np.float32)
    kv = rng.standard_normal((N, P2, W2, C_KV), dtype=np.float32)
    out = kernel(r_idx, r_weight, kv)
    # local reference
    bidx = np.arange(N)[:, None, None]
    exp = r_weight[..., None, None] * kv[bidx, r_idx]
    err = np.abs(out - exp).max() / (np.abs(exp).max() + 1e-30)
    print("abs-rel err:", err)


# revision 15
# speedup vs baseline: 6.0658x; 2.1918x over previous
"""KVGather Trainium2 kernel.

Problem: out[n, i, k] = r_weight[n, i, k] * kv[n, r_idx[n, i, k]]
  r_idx:    (16, 64, 8)  int64, values in [0, 64)
  r_weight: (16, 64, 8)  float32
  kv:       (16, 64, 64, 128) float32
  out:      (16, 64, 8, 64, 128) float32

Strategy: data-parallel over batch n across 8 NeuronCores (2 batches/core).
Per core: 4 MB kv in, 32 MB out -> HBM-write-bound (~100us at ~358 GB/s).

The gather+scale is reformulated as a one-hot matmul so all device
addressing is static:
  - Host splits f32 kv into kv_hi + kv_lo (bf16 pair, exact to ~2^-18)
    and packs them per batch into one [128, F] plane: partitions 0..63
    hold kv_hi regions, 64..127 hold kv_lo regions.
  - Host builds selection matrices S (bf16): column o (an output slot)
    has TWO ones - at row r_o and row r_o+64 - so a single bf16 matmul
    accumulates hi+lo in fp32 PSUM: psum[o,f] = hi[r_o,f] + lo[r_o,f].
  - DVE/ACT drain PSUM -> SBUF fused with the full-precision f32 weight
    multiply (1024-wide ops spanning two PSUM banks).
  - SWDGE DMA (gpsimd) streams staging -> out DRAM in 1MB transfers
    across all 16 SDMA engines.
"""

import sys

for _p in ("/opt/trn_rl_repo",):
    if _p not in sys.path:
        sys.path.insert(0, _p)

import numpy as np
import ml_dtypes

from concourse import bass, bacc, tile
from concourse import mybir
from concourse.bass_utils import run_bass_kernel_spmd

# Problem constants (hardcoded per contract)
N, P2, TOPK, W2, C_KV = 16, 64, 8, 64, 128
N_CORES = 8
B = N // N_CORES            # batches per core = 2
SLOTS = P2 * TOPK           # 512 output slots per batch
F = W2 * C_KV               # 8192 elements per region
N_CHUNK = 2 * SLOTS // 128  # 8 chunks of 128 packed output slots
FC = 4                      # kv f-dim split for load/compute overlap
F_PER_FC = F // FC          # 2048
T_PER_FC = F_PER_FC // 512  # 4 psum tiles of 512 per f-chunk

_cached = {}


def _build_program():
    """Build the (input-independent) Bass program once."""
    if "nc" in _cached:
        return _cached["nc"]

    bf16 = mybir.dt.bfloat16
    f32 = mybir.dt.float32

    nc = bacc.Bacc()

    # Per-core inputs. kv packed per batch b into [128, FC, F_PER_FC]:
    # partition p in [0,64) = kv_hi region p; p in [64,128) = kv_lo
    # region p-64; free = region elems (f-chunked).
    kv_d = [
        nc.dram_tensor(f"kv{b}", [128, FC, F_PER_FC], bf16, kind="ExternalInput")
        for b in range(B)
    ]
    # Selection matrices, r-major: s_d[r, c, o] for chunk c = (b, jj):
    # column o (slot jj*128+o of batch b) has ones at rows r_o and r_o+64.
    s_d = nc.dram_tensor("sel", [128, N_CHUNK, 128], bf16, kind="ExternalInput")
    # w_d[o, c] = f32 weight of chunk c's slot o.
    w_d = nc.dram_tensor("wgt", [128, N_CHUNK], f32, kind="ExternalInput")
    # Output: [b, slot(512), F] - each chunk's store is 128 slots x F-range.
    out_d = nc.dram_tensor("out", [B, SLOTS, F], f32, kind="ExternalOutput")

    with tile.TileContext(nc) as tc:
        with (
            tc.tile_pool(name="const", bufs=1) as const_pool,
            tc.tile_pool(name="kv", bufs=1) as kv_pool,
            tc.tile_pool(name="stage", bufs=6) as stage_pool,
            tc.tile_pool(name="psum", bufs=4, space=bass.MemorySpace.PSUM) as psum_pool,
        ):
            s_sb = const_pool.tile([128, N_CHUNK, 128], bf16, tag="sel")
            w_sb = const_pool.tile([128, N_CHUNK], f32, tag="wgt")
            nc.gpsimd.dma_start(out=s_sb[:], in_=s_d[:])
            nc.gpsimd.dma_start(out=w_sb[:], in_=w_d[:])

            # kv planes as separate tiles so compute can start after the
            # first f-chunk of batch 0 lands.
            kv_sb = {}
            for b in range(B):
                for fc in range(FC):
                    tkv = kv_pool.tile([128, F_PER_FC], bf16, tag=f"kv{b}_{fc}")
                    nc.gpsimd.dma_start(out=tkv[:], in_=kv_d[b][:, fc, :])
                    kv_sb[(b, fc)] = tkv

            group_i = 0
            JJ = SLOTS // 128  # 4 chunks of 128 slots per batch
            for b in range(B):
                for fc in range(FC):
                    for jj in range(JJ):
                        c = b * JJ + jj  # chunk id
                        # One engine per group keeps per-instruction sync
                        # waits within the ISA budget; groups alternate
                        # DVE/ACT so both engines split the drain work.
                        use_dve = group_i % 2 == 0
                        group_i += 1
                        stage = stage_pool.tile([128, F_PER_FC], f32, tag="stage")
                        for th in range(T_PER_FC // 2):
                            # 2-bank PSUM tile; two 512-wide matmuls fill it,
                            # one 1024-wide op drains it.
                            ps = psum_pool.tile([128, 1024], f32, tag="ps")
                            for h in range(2):
                                t = th * 2 + h
                                nc.tensor.matmul(
                                    ps[:, h * 512 : (h + 1) * 512],
                                    s_sb[:, c, :],
                                    kv_sb[(b, fc)][:, t * 512 : (t + 1) * 512],
                                    start=True,
                                    stop=True,
                                )
                            sl = stage[:, th * 1024 : (th + 1) * 1024]
                            if use_dve:
                                nc.vector.tensor_mul(
                                    sl,
                                    ps[:],
                                    w_sb[:, c : c + 1].broadcast_to([128, 1024]),
                                )
                            else:
                                nc.scalar.activation(
                                    sl,
                                    ps[:],
                                    mybir.ActivationFunctionType.Copy,
                                    scale=w_sb[:, c : c + 1],
                                )
                        # Contiguous store: 128 slots x F_PER_FC slice (1MB).
                        nc.gpsimd.dma_start(
                            out=out_d[
                                b,
                                jj * 128 : (jj + 1) * 128,
                                fc * F_PER_FC : (fc + 1) * F_PER_FC,
                            ],
                            in_=stage[:],
                        )

    nc.compile()
    _cached["nc"] = nc
    return nc


def _prep_inputs(r_idx, r_weight, kv):
    """Shard + transform host inputs into per-core in_maps."""
    r_idx = np.asarray(r_idx).astype(np.int64)
    r_weight = np.asarray(r_weight).astype(np.float32)
    kv = np.asarray(kv).astype(np.float32)

    # hi/lo bf16 split of kv (exact to ~2^-18 relative)
    kv_hi = kv.astype(ml_dtypes.bfloat16)
    kv_lo = (kv - kv_hi.astype(np.float32)).astype(ml_dtypes.bfloat16)

    JJ = SLOTS // 128
    in_maps = []
    for m in range(N_CORES):
        bsl = slice(m * B, (m + 1) * B)
        idx = r_idx[bsl].reshape(B, SLOTS)        # [2, 512] region ids
        wgt = r_weight[bsl].reshape(B, SLOTS)     # [2, 512] f32

        im = {}
        for b in range(B):
            # [64, F] hi over [64, F] lo -> [128, F] -> [128, FC, F_PER_FC]
            plane = np.concatenate(
                [
                    kv_hi[m * B + b].reshape(P2, F),
                    kv_lo[m * B + b].reshape(P2, F),
                ],
                axis=0,
            )
            im[f"kv{b}"] = np.ascontiguousarray(plane.reshape(128, FC, F_PER_FC))

        # S[r, c, o]: chunk c=(b,jj); slot jj*128+o of batch b routes
        # region r_o: ones at rows r_o (hi) and r_o+64 (lo).
        S = np.zeros((128, N_CHUNK, 128), dtype=ml_dtypes.bfloat16)
        W = np.zeros((128, N_CHUNK), dtype=np.float32)
        for b in range(B):
            for jj in range(JJ):
                c = b * JJ + jj
                slots = np.arange(jj * 128, (jj + 1) * 128)
                r = idx[b, slots]
                S[r, c, np.arange(128)] = 1.0
                S[r + 64, c, np.arange(128)] = 1.0
                W[:, c] = wgt[b, slots]

        im["sel"] = S
        im["wgt"] = W
        in_maps.append(im)
    return in_maps


def _ensure_ntff_hook():
    """The agent image's antenv lacks axon_hooks, so the boot-time NTFF
    hook registration silently no-ops. Recreate the module and register
    the ctypes hook so trace=True yields exec_time_ns."""
    import types
    import antenv

    if "antenv.axon_hooks" in sys.modules:
        return
    mod = types.ModuleType("antenv.axon_hooks")
    _state = {"hook": None}
    mod.set_axon_ntff_profile_hook = lambda h: _state.__setitem__("hook", h)
    mod.get_axon_ntff_profile_hook = lambda: _state["hook"]
    sys.modules["antenv.axon_hooks"] = mod
    antenv.axon_hooks = mod
    try:
        if "/root/.axon_site" not in sys.path:
            sys.path.insert(0, "/root/.axon_site")
        from trn_agent_boot.trn_boot import _ntff_profile_via_ctypes

        hook = _ntff_profile_via_ctypes("/opt/axon/libaxon_pjrt.so")
        if hook is not None:
            mod.set_axon_ntff_profile_hook(hook)
    except Exception:
        pass


def kernel(r_idx, r_weight, kv, _trace=False, _trace_kwargs=None):
    if _trace:
        _ensure_ntff_hook()
    nc = _build_program()
    in_maps = _prep_inputs(r_idx, r_weight, kv)
    res = run_bass_kernel_spmd(
        nc,
        in_maps,
        core_ids=list(range(N_CORES)),
        trace=_trace,
        **(_trace_kwargs or {}),
    )
    out = np.empty((N, P2, TOPK, W2, C_KV), dtype=np.float32)
    for m in range(N_CORES):
        o = res.results[m]["out"]  # [B, SLOTS, F]
        out[m * B : (m + 1) * B] = o.reshape(B, P2, TOPK, W2, C_KV)
    if _trace:
        return out, res
    return out


if __name__ == "__main__":
    rng = np.random.default_rng(0)
    r_idx = rng.integers(0, P2, (N, P2, TOPK)).astype(np.int64)
    r_weight = rng.random((N, P2, TOPK), dtype=np.float32)
    kv = rng.standard_normal((N, P2, W2, C_KV), dtype=np.float32)
    out = kernel(r_idx, r_weight, kv)
    # local reference
    bidx = np.arange(N)[:, None, None]
    exp = r_weight[..., None, None] * kv[bidx, r_idx]
    err = np.abs(out - exp).max() / (np.abs(exp).max() + 1e-30)
    print("abs-rel err:", err)
